# revision 2
# baseline (speedup 1.0000x reference)
"""PatchCore (retrieval kNN) kernel for 8 Trainium2 NeuronCores.

Strategy (per spec sharding_hint): shard the 50000-row memory bank across the
8 cores (6250 rows each). Each core computes, for every one of the 6272
embedding rows, min over its bank shard of (||y||^2/2 - x.y) via a bf16
matmul (fp32 PSUM accumulate) + DVE subtract + min-reduce. The host merges
the 8 partial mins (all-reduce min), reconstructs min squared distances as
xnorm + 2*min_w, and runs the tiny downstream PatchCore epilogue (per-batch
max patch, 9-NN reweighting, bilinear upsample + gaussian blur) in fp32
numpy, exactly mirroring the reference.

Self-contained: hardcodes all shapes; only imports installed packages
(numpy, ml_dtypes, jax, concourse).
"""

import sys

if "/opt/trn_rl_repo" not in sys.path:
    sys.path.insert(0, "/opt/trn_rl_repo")

import numpy as np
import ml_dtypes

# ---- problem constants -----------------------------------------------------
B = 8
P = 28
D = 1536
M = 50000
N = B * P * P            # 6272 embedding rows
NCORES = 8
MS = M // NCORES         # 6250 bank rows per core
KT = D // 128            # 12 contraction (k) tiles
NT = N // 128            # 49 embedding row tiles
MTILE = 512
M_TILES = [(j * MTILE, min(MTILE, MS - j * MTILE)) for j in range((MS + MTILE - 1) // MTILE)]
NJ = len(M_TILES)        # 13
NUM_NEIGHBORS = 9
INPUT_SIZE = 224
SIGMA = 4.0
KSIZE = 2 * int(4.0 * SIGMA + 0.5) + 1   # 33

_RT = {}  # cached runtime: nc, jitted runner, etc.


# ---- device kernel ---------------------------------------------------------

def _build_nc():
    import concourse.bacc as bacc
    from concourse import mybir, tile

    F32 = mybir.dt.float32
    BF16 = mybir.dt.bfloat16
    SUB = mybir.AluOpType.subtract
    MIN = mybir.AluOpType.min
    AX = mybir.AxisListType.X

    nc = bacc.Bacc(None, target_bir_lowering=False)

    embT = nc.dram_tensor("embT", [KT, 128, N], BF16, kind="ExternalInput")
    bankT = nc.dram_tensor("bankT", [KT, 128, MS], BF16, kind="ExternalInput")
    ynb = nc.dram_tensor("ynb", [128, MS], F32, kind="ExternalInput")
    out = nc.dram_tensor("out", [128, NT], F32, kind="ExternalOutput")

    with tile.TileContext(nc) as tc:
        with (
            tc.tile_pool(name="emb", bufs=1) as embp,
            tc.tile_pool(name="bank", bufs=2) as bankp,
            tc.tile_pool(name="yn", bufs=2) as ynp,
            tc.tile_pool(name="w", bufs=3) as wp,
            tc.tile_pool(name="tmin", bufs=1) as tminp,
            tc.tile_pool(name="outp", bufs=1) as outp,
            tc.tile_pool(name="psum", bufs=3, space="PSUM") as psump,
        ):
            # embedding resident in SBUF for the whole kernel
            emb_tiles = []
            for d in range(KT):
                t = embp.tile([128, N], BF16, tag=f"e{d}")
                nc.sync.dma_start(t[:], embT[d])
                emb_tiles.append(t)

            # per-(n_tile, m_tile) minima; reduced at the end
            tmin = tminp.tile([128, NT * NJ], F32)
            outb = outp.tile([128, NT], F32)

            for j, (m0, mw) in enumerate(M_TILES):
                bts = []
                for d in range(KT):
                    t = bankp.tile([128, MTILE], BF16, tag=f"b{d}")
                    nc.sync.dma_start(t[:, :mw], bankT[d, :, m0:m0 + mw])
                    bts.append(t)
                ynt = ynp.tile([128, MTILE], F32, tag="yn")
                nc.sync.dma_start(ynt[:, :mw], ynb[:, m0:m0 + mw])

                for nt in range(NT):
                    ps = psump.tile([128, MTILE], F32, tag="ps")
                    for d in range(KT):
                        nc.tensor.matmul(
                            ps[:, :mw],
                            emb_tiles[d][:, nt * 128:(nt + 1) * 128],
                            bts[d][:, :mw],
                            start=(d == 0),
                            stop=(d == KT - 1),
                        )
                    wt = wp.tile([128, MTILE], F32, tag="w")
                    nc.vector.tensor_tensor(
                        out=wt[:, :mw], in0=ynt[:, :mw], in1=ps[:, :mw], op=SUB
                    )
                    nc.vector.tensor_reduce(
                        out=tmin[:, nt * NJ + j: nt * NJ + j + 1],
                        in_=wt[:, :mw], op=MIN, axis=AX,
                    )

            nc.vector.tensor_reduce(
                out=outb[:],
                in_=tmin[:].rearrange("p (a b) -> p a b", b=NJ),
                op=MIN, axis=AX,
            )
            nc.sync.dma_start(out[:], outb[:])

    nc.compile()
    return nc


def _get_runtime():
    """Build + compile the bass kernel and a cached jitted 8-core runner."""
    if _RT:
        return _RT

    import jax
    from jax.experimental.shard_map import shard_map
    from jax.sharding import Mesh, PartitionSpec
    from concourse import mybir
    from concourse import bass2jax

    bass2jax.install_neuronx_cc_hook()
    nc = _build_nc()

    partition_name = nc.partition_id_tensor.name if nc.partition_id_tensor else None
    in_names, out_names, out_avals, zero_outs = [], [], [], []
    for alloc in nc.m.functions[0].allocations:
        if not isinstance(alloc, mybir.MemoryLocationSet):
            continue
        name = alloc.memorylocations[0].name
        if alloc.kind == "ExternalInput":
            if name != partition_name:
                in_names.append(name)
        elif alloc.kind == "ExternalOutput":
            shape = tuple(alloc.tensor_shape)
            dtype = mybir.dt.np(alloc.dtype)
            out_names.append(name)
            out_avals.append(jax.core.ShapedArray(shape, dtype))
            zero_outs.append(np.zeros(shape, dtype))
    n_params = len(in_names)
    n_outs = len(out_avals)
    all_in_names = list(in_names) + list(out_names)
    if partition_name is not None:
        all_in_names.append(partition_name)

    def _body(*args):
        operands = list(args)
        if partition_name is not None:
            operands.append(bass2jax.partition_id_tensor())
        outs = bass2jax._bass_exec_p.bind(
            *operands,
            out_avals=tuple(out_avals),
            in_names=tuple(all_in_names),
            out_names=tuple(out_names),
            lowering_input_output_aliases=(),
            sim_require_finite=True,
            sim_require_nnan=True,
            nc=nc,
        )
        return tuple(outs)

    devices = jax.devices()[:NCORES]
    mesh = Mesh(np.asarray(devices), ("core",))
    in_specs = (PartitionSpec("core"),) * (n_params + n_outs)
    out_specs = (PartitionSpec("core"),) * n_outs
    donate = tuple(range(n_params, n_params + n_outs))
    fn = jax.jit(
        shard_map(_body, mesh=mesh, in_specs=in_specs, out_specs=out_specs,
                  check_rep=False),
        donate_argnums=donate,
        keep_unused=True,
    )

    _RT.update(
        nc=nc, fn=fn, mesh=mesh, in_names=in_names, out_names=out_names,
        out_avals=out_avals, zero_outs=zero_outs, jax=jax,
        PartitionSpec=PartitionSpec,
    )
    return _RT


def _run_device(in_maps):
    """Run the compiled kernel on 8 cores. in_maps: per-core dict name->array.
    Returns list of per-core output dicts, and caches device inputs for
    bench_ns()."""
    rt = _get_runtime()
    jax = rt["jax"]
    from jax.sharding import NamedSharding

    concat_in = [
        np.concatenate([np.asarray(in_maps[c][name]) for c in range(NCORES)], axis=0)
        for name in rt["in_names"]
    ]
    sharding = NamedSharding(rt["mesh"], rt["PartitionSpec"]("core"))
    dev_in = [jax.device_put(a, sharding) for a in concat_in]
    rt["dev_in"] = dev_in

    concat_zeros = [
        np.zeros((NCORES * z.shape[0], *z.shape[1:]), z.dtype)
        for z in rt["zero_outs"]
    ]
    out_arrs = rt["fn"](*dev_in, *concat_zeros)
    out_arrs = [np.asarray(a) for a in out_arrs]
    return [
        {
            name: out_arrs[i].reshape(NCORES, *rt["out_avals"][i].shape)[c]
            for i, name in enumerate(rt["out_names"])
        }
        for c in range(NCORES)
    ]


def bench_ns(iters=10):
    """Amortized per-iteration device execution time (ns) with inputs already
    resident on device. Call kernel() first."""
    import time
    rt = _get_runtime()
    assert "dev_in" in rt, "call kernel() before bench_ns()"
    jax = rt["jax"]
    dev_in = rt["dev_in"]

    def one():
        zeros = [
            np.zeros((NCORES * z.shape[0], *z.shape[1:]), z.dtype)
            for z in rt["zero_outs"]
        ]
        return rt["fn"](*dev_in, *zeros)

    # warmup
    r = one()
    jax.block_until_ready(r)
    t0 = time.perf_counter()
    rs = [one() for _ in range(iters)]
    jax.block_until_ready(rs)
    t1 = time.perf_counter()
    return (t1 - t0) / iters * 1e9


# ---- host epilogue (exact fp32 numpy mirror of the reference tail) ---------

def _resize_weight_mat(in_size, out_size):
    """jax.image.resize 'bilinear' weight matrix [in_size, out_size]
    (upsampling; antialias irrelevant)."""
    inv_scale = in_size / out_size
    sample_f = (np.arange(out_size, dtype=np.float64) + 0.5) * inv_scale - 0.5
    x = np.abs(sample_f[None, :] - np.arange(in_size, dtype=np.float64)[:, None])
    weights = np.maximum(0.0, 1.0 - x)
    total = weights.sum(axis=0, keepdims=True)
    weights = np.where(np.abs(total) > 1000.0 * np.finfo(np.float32).eps,
                       weights / np.where(total != 0, total, 1), 0.0)
    weights = np.where(
        ((sample_f >= -0.5) & (sample_f <= in_size - 0.5))[None, :], weights, 0.0)
    return weights.astype(np.float32)


def _gaussian_kernel1d(ksize, sigma):
    x = np.arange(ksize, dtype=np.float32) - (ksize - 1) / 2.0
    g = np.exp(-(x * x) / (2.0 * sigma * sigma))
    return g / g.sum()


def _blur_mat(out_size, ksize, sigma):
    """[out_size, out_size + ksize - 1] VALID-conv matrix with gaussian taps."""
    g = _gaussian_kernel1d(ksize, sigma)
    m = np.zeros((out_size, out_size + ksize - 1), np.float32)
    for i in range(out_size):
        m[i, i:i + ksize] = g
    return m


def _softmax(x, axis=-1):
    x = x - x.max(axis=axis, keepdims=True)
    e = np.exp(x)
    return e / e.sum(axis=axis, keepdims=True)


def _epilogue(patch_scores_flat, embedding, memory_bank, ynorm):
    """From per-row min distances to (anomaly_map, pred_score); all fp32."""
    patch_scores = patch_scores_flat.reshape(B, P * P)

    # per-batch max patch
    max_patches = patch_scores.argmax(axis=1)                      # [B]
    score = patch_scores[np.arange(B), max_patches]                # [B]
    rows = np.arange(B) * (P * P) + max_patches
    feats = embedding[rows]                                        # [B, D]

    # exact nearest neighbor of the max patches (argmin over full bank)
    fnorm = (feats * feats).sum(1)                                 # [B]
    d2_rows = fnorm[:, None] - 2.0 * (feats @ memory_bank.T) + ynorm[None, :]
    nn_index = d2_rows.argmin(axis=1)                              # [B]
    nn_sample = memory_bank[nn_index]                              # [B, D]

    # 9 nearest neighbors of nn_sample
    nnorm = (nn_sample * nn_sample).sum(1)
    d2 = nnorm[:, None] - 2.0 * (nn_sample @ memory_bank.T) + ynorm[None, :]
    d2 = np.sqrt(np.clip(d2, 0.0, None))                           # [B, M] distances
    support = np.argsort(d2, axis=1, kind="stable")[:, :NUM_NEIGHBORS]  # [B, 9]

    supp_feats = memory_bank[support]                              # [B, 9, D]
    sf_norm = (supp_feats * supp_feats).sum(-1)                    # [B, 9]
    d3 = fnorm[:, None] - 2.0 * np.einsum("bd,bkd->bk", feats, supp_feats) + sf_norm
    d3 = np.sqrt(np.clip(d3, 0.0, None))                           # [B, 9]
    weights = 1.0 - _softmax(d3, axis=1)[:, 0]                     # [B]
    pred_score = (weights * score).astype(np.float32)

    # anomaly map: bilinear 28->224 then gaussian blur with reflect pad
    pmap = patch_scores.reshape(B, 1, P, P).astype(np.float32)
    W = _resize_weight_mat(P, INPUT_SIZE)                          # [28, 224]
    amap = np.einsum("bchw,hH,wW->bcHW", pmap, W, W, optimize=True)
    pad = KSIZE // 2
    padded = np.pad(amap, ((0, 0), (0, 0), (pad, pad), (pad, pad)), mode="reflect")
    Bm = _blur_mat(INPUT_SIZE, KSIZE, SIGMA)                       # [224, 256]
    t1 = np.einsum("Hh,bchw->bcHw", Bm, padded, optimize=True)
    anomaly_map = np.einsum("Ww,bcHw->bcHW", Bm, t1, optimize=True)
    return anomaly_map.astype(np.float32), pred_score


# ---- public entry ----------------------------------------------------------

def kernel(embedding, memory_bank):
    embedding = np.ascontiguousarray(np.asarray(embedding, dtype=np.float32))
    memory_bank = np.ascontiguousarray(np.asarray(memory_bank, dtype=np.float32))
    assert embedding.shape == (N, D) and memory_bank.shape == (M, D)

    ynorm = (memory_bank * memory_bank).sum(1).astype(np.float32)   # [M]
    xnorm = (embedding * embedding).sum(1).astype(np.float32)       # [N]

    # per-core inputs: bank shard transposed, embedding transposed (replicated)
    embT = np.ascontiguousarray(embedding.T).reshape(KT, 128, N).astype(ml_dtypes.bfloat16)
    in_maps = []
    for c in range(NCORES):
        shard = memory_bank[c * MS:(c + 1) * MS]                    # [MS, D]
        bankT = np.ascontiguousarray(shard.T).reshape(KT, 128, MS).astype(ml_dtypes.bfloat16)
        yh = 0.5 * ynorm[c * MS:(c + 1) * MS]                       # [MS]
        ynb = np.ascontiguousarray(np.broadcast_to(yh[None, :], (128, MS))).astype(np.float32)
        in_maps.append({"embT": embT, "bankT": bankT, "ynb": ynb})

    results = _run_device(in_maps)

    # merge: out[p, nt] holds min_w for row n = nt*128 + p
    per_core = np.stack([r["out"].T.reshape(N) for r in results])   # [8, N]
    min_w = per_core.min(axis=0)                                    # [N]
    d2min = xnorm + 2.0 * min_w
    patch_scores_flat = np.sqrt(np.clip(d2min, 0.0, None)).astype(np.float32)

    return _epilogue(patch_scores_flat, embedding, memory_bank, ynorm)
